# revision 60
# baseline (speedup 1.0000x reference)
"""Distributed Trainium2 kernel for a dense-transformer attention block.

Math (matches the reference):
    xqkv = x @ Wqkv + bqkv ; split into q,k,v heads
    scores = (q k^T) / sqrt(HD) + mask ; attn = softmax(scores)
    o = attn @ v ; out = o @ Wproj + bproj

Parallelization over 8 NeuronCores:
  - QKV projection is DATA-parallel: each core projects its own 1/8 of
    the tokens against the FULL Wqkv (same FLOPs as a head-parallel
    projection, but only 1/8 of x ever needs the dma-transpose and
    there is no big x AllGather).  An AllToAll then regroups q/k/v so
    each core holds 2 heads over ALL tokens (head-parallel attention).
  - After attention a second AllToAll redistributes the per-head
    outputs so each core holds ALL head-dims for its 1/8 of the rows
    and runs the output projection; the host concatenates row blocks.

Layout notes:
  - q/k are produced transposed ([head_dim, token]) so TensorE can
    contract over the model dim; v is produced in natural layout.
  - Scores are computed transposed (s^T[kv, q]) so softmax row-sums are
    matmuls with an all-ones stationary tile (which also broadcasts the
    sums across partitions) and attn@v consumes p^T directly.
  - Softmax skips max-subtraction (|scores| <= ~8 for this problem).
  - The additive mask is analyzed on the host: fully-masked 512x512
    chunks are skipped, fully-visible chunks run unmasked, and mixed
    chunks get (deduplicated) mask tiles added to the score PSUM.
"""

import hashlib
import numpy as np
import ml_dtypes

B, S, DIM, NH = 4, 2048, 2048, 16
HD = DIM // NH  # 128
NCORES = 8
HPC = NH // NCORES          # heads per core = 2
TOK = B * S                 # 8192 tokens
RPC = TOK // NCORES         # rows (tokens) per core = 1024
CH = 512                    # attention chunk (q and kv)
SUB = 128                   # kv subtile
SCALE = 1.0 / float(np.sqrt(HD))

_BF16 = ml_dtypes.bfloat16

_prog_cache = {}


def _analyze_mask(mask):
    """Build the attention schedule from the additive mask.

    sched[qc] = list of (kc, j, q_lo, mask_id, c_lo, c_hi); mask_id is
    -1 when no mask add is needed for the entry.  Mask tiles are already
    transposed to [kv, q] layout and pre-divided by SCALE.
    """
    m = np.asarray(mask, dtype=np.float32).reshape(S, S)
    NEG = -1e8
    sched = []
    tiles = []
    tile_key = {}
    for qc in range(S // CH):
        ents = []
        for kc in range(S // CH):
            blk = m[qc * CH:(qc + 1) * CH, kc * CH:(kc + 1) * CH]
            if np.all(blk <= NEG):
                continue
            for j in range(CH // SUB):
                sub = blk[:, j * SUB:(j + 1) * SUB]       # [CH q, SUB kv]
                if np.all(sub <= NEG):
                    continue
                vis = ~np.all(sub <= NEG, axis=1)
                q_lo = int(np.argmax(vis))
                q_lo = (q_lo // SUB) * SUB
                if not ents:
                    q_lo = 0  # first entry must initialize full PSUM width
                nzrow = np.any(sub[q_lo:, :] != 0.0, axis=1)
                if nzrow.any():
                    first = q_lo + int(np.argmax(nzrow))
                    last = q_lo + len(nzrow) - int(np.argmax(nzrow[::-1]))
                    c_lo = (first // SUB) * SUB
                    c_hi = min(CH, ((last + SUB - 1) // SUB) * SUB)
                    content = np.ascontiguousarray(
                        (sub[c_lo:c_hi, :].T / SCALE).astype(_BF16))
                    key = (c_hi - c_lo,
                           hashlib.md5(content.tobytes()).hexdigest())
                    if key not in tile_key:
                        tile_key[key] = len(tiles)
                        tiles.append(content)
                    ents.append((kc, j, q_lo, tile_key[key], c_lo, c_hi))
                else:
                    ents.append((kc, j, q_lo, -1, 0, 0))
        assert ents, "a full query chunk is masked out; softmax undefined"
        sched.append(ents)
    n_real = len(tiles)
    widths = [t.shape[1] for t in tiles]
    pack = np.zeros((max(1, n_real), SUB, CH), dtype=_BF16)
    for i, t in enumerate(tiles):
        pack[i, :, :t.shape[1]] = t
    return sched, pack, widths, n_real


def _build_program(sched, n_mask_tiles, mask_widths):
    import concourse.bass as bass
    import concourse.tile as tile
    from concourse import bacc, bass_isa, mybir
    from contextlib import ExitStack

    f32 = mybir.dt.float32
    bf16 = mybir.dt.bfloat16
    AF = mybir.ActivationFunctionType
    ALU = mybir.AluOpType

    nc = bacc.Bacc("TRN2", target_bir_lowering=False, debug=False,
                   num_devices=NCORES)

    xsT_ext = nc.dram_tensor("xsT", [DIM, RPC], bf16,
                             kind="ExternalInput").ap()
    wqk_ext = nc.dram_tensor("wqk", [DIM, 2 * DIM], bf16,
                             kind="ExternalInput").ap()
    wv_ext = nc.dram_tensor("wv", [DIM, DIM], bf16,
                            kind="ExternalInput").ap()
    bqk_ext = nc.dram_tensor("bqk", [2 * DIM, 1], f32,
                             kind="ExternalInput").ap()
    bv_ext = nc.dram_tensor("bv", [1, DIM], f32, kind="ExternalInput").ap()
    maskt_ext = nc.dram_tensor("maskt", [max(1, n_mask_tiles), SUB, CH], bf16,
                               kind="ExternalInput").ap()
    wproj_ext = nc.dram_tensor("wproj", [DIM, DIM], bf16,
                               kind="ExternalInput").ap()
    bproj_ext = nc.dram_tensor("bproj", [1, DIM], bf16,
                               kind="ExternalInput").ap()
    out_ext = nc.dram_tensor("out", [RPC, DIM], bf16,
                             kind="ExternalOutput").ap()

    NDT = DIM // 128          # 16 contraction tiles
    NSC = S // CH             # 4 s-chunks per batch
    NQKM = 2 * DIM // 128     # 32 q/k output M-tiles (dest-grouped)
    rg = [list(range(NCORES))]

    with tile.TileContext(nc) as tc, ExitStack() as top:
        dram = top.enter_context(tc.tile_pool(name="dram", bufs=1,
                                              space="DRAM"))
        q_send = dram.tile([DIM, RPC], bf16, name="q_send")
        k_sendA = dram.tile([NCORES * HD, RPC], bf16, name="k_sendA")
        k_sendB = dram.tile([NCORES * HD, RPC], bf16, name="k_sendB")
        v_send = dram.tile([TOK, HPC * HD], bf16, name="v_send")
        q_out = dram.tile([DIM, RPC], bf16, name="q_out")
        k_outA = dram.tile([NCORES * HD, RPC], bf16, name="k_outA")
        k_outB = dram.tile([NCORES * HD, RPC], bf16, name="k_outB")
        v_out = dram.tile([TOK, HPC * HD], bf16, name="v_out")
        a2a_in = [dram.tile([DIM, RPC // 2], bf16, name=f"a2a_in{i}")
                  for i in range(2)]
        a2a_out = [dram.tile([DIM, RPC // 2], bf16, name=f"a2a_out{i}")
                   for i in range(2)]

        const = top.enter_context(tc.tile_pool(name="const", bufs=1))
        ones = const.tile([128, 128], bf16, name="ones", tag="ones")
        nc.any.memset(ones[:], 1.0)
        bp1 = const.tile([1, DIM], bf16, name="bp1", tag="bp1")
        nc.sync.dma_start(out=bp1[:], in_=bproj_ext[:, :])
        msk = []
        for i in range(n_mask_tiles):
            w = mask_widths[i]
            mt = const.tile([128, w], bf16, name=f"msk{i}", tag=f"msk{i}")
            nc.sync.dma_start(out=mt[:], in_=maskt_ext[i, :, :w])
            msk.append(mt)


        # ================= Phase 1: data-parallel QKV projection ======
        with ExitStack() as p1:
            psA = p1.enter_context(tc.tile_pool(name="psA", bufs=4,
                                                space="PSUM"))
            psV = p1.enter_context(tc.tile_pool(name="psV", bufs=4,
                                                space="PSUM"))
            # x^T tiles (host pre-transposed): [dim-subtile 128, tok-chunk]
            # chunk 0 loads on the sync queue, chunk 1 on gpsimd, so the
            # first M-tile's matmuls can start within a few microseconds.
            xtp = p1.enter_context(tc.tile_pool(name="xtp", bufs=1))
            xT = [[None] * NDT for _ in range(RPC // CH)]
            for nchk in range(RPC // CH):
                eng = nc.sync if nchk == 0 else nc.gpsimd
                for dt_i in range(NDT):
                    xt = xtp.tile([128, CH], bf16, name=f"xT{nchk}_{dt_i}",
                                  tag=f"xT{nchk}_{dt_i}")
                    eng.dma_start(
                        out=xt[:],
                        in_=xsT_ext[dt_i * 128:(dt_i + 1) * 128,
                                    nchk * CH:(nchk + 1) * CH])
                    xT[nchk][dt_i] = xt

            bqkp = p1.enter_context(tc.tile_pool(name="bqkp", bufs=8))
            wqkp = p1.enter_context(tc.tile_pool(name="wqkp", bufs=8))
            # deep store buffering: the send-buffer stores contend with the
            # in-flight AllToAll on the wire; 16 tiles of slack (~67us of
            # matmul issue) lets TensorE ride out a slow collective
            qksb = p1.enter_context(tc.tile_pool(name="qksb", bufs=16))

            def qk_tile(m):
                # one dest-grouped M-tile of [dim, 128]; m<16: q,
                # 16..23: k head-0 half, 24..31: k head-1 half
                # weight/bias loads ride the scalar queue: it is idle during
                # QKV, so the first matmul isn't gated behind the x loads
                wm = wqkp.tile([128, DIM], bf16, name="wm", tag="wm")
                nc.scalar.dma_start(
                    out=wm[:].rearrange("p (dt c) -> p dt c", dt=NDT),
                    in_=wqk_ext[:, m * 128:(m + 1) * 128].rearrange(
                        "(dt p) c -> p dt c", p=128))
                bm = bqkp.tile([128, 1], f32, name="bm", tag="bm")
                nc.scalar.dma_start(out=bm[:],
                                    in_=bqk_ext[m * 128:(m + 1) * 128, :])
                if m < NQKM // 2:
                    dst, mm = q_send, m
                elif m < 3 * NQKM // 4:
                    dst, mm = k_sendA, m - NQKM // 2
                else:
                    dst, mm = k_sendB, m - 3 * NQKM // 4
                for nchk in range(RPC // CH):
                    ps = psA.tile([128, CH], f32, name="psqk", tag="A")
                    for dt_i in range(NDT):
                        nc.tensor.matmul(
                            ps[:], wm[:, dt_i * 128:(dt_i + 1) * 128],
                            xT[nchk][dt_i][:],
                            start=(dt_i == 0), stop=(dt_i == NDT - 1))
                    sb = qksb.tile([128, CH], bf16, name="sb", tag="sb")
                    nc.vector.tensor_scalar_add(sb[:], ps[:], bm[:])
                    nc.sync.dma_start(
                        out=dst[mm * 128:(mm + 1) * 128,
                                nchk * CH:(nchk + 1) * CH],
                        in_=sb[:])

            # v runs FIRST: it has the heaviest store/DVE pipeline (64
            # v_send stores), and doing it before any collective is in
            # flight keeps those stores off the A2A-congested wire.
            bvp = p1.enter_context(tc.tile_pool(name="bvp", bufs=1))
            bv1 = bvp.tile([1, DIM], f32, name="bv1", tag="bv1")
            nc.sync.dma_start(out=bv1[:], in_=bv_ext[:, :])
            bvb = bvp.tile([128, DIM], f32, name="bvb", tag="bvb")
            nc.gpsimd.partition_broadcast(bvb[:], bv1[:])
            wvp = p1.enter_context(tc.tile_pool(name="wvp", bufs=2))
            vsb = p1.enter_context(tc.tile_pool(name="vsb", bufs=16))

            def load_wv(dhc):
                tiles = []
                for dt_i in range(NDT):
                    wv = wvp.tile([128, CH], bf16, name=f"wv{dhc}_{dt_i}",
                                  tag=f"wv{dt_i}")
                    # scalar queue: lands ahead of the sync-queue x loads,
                    # so the first v matmul can issue within ~7us
                    nc.scalar.dma_start(
                        out=wv[:],
                        in_=wv_ext[dt_i * 128:(dt_i + 1) * 128,
                                   dhc * CH:(dhc + 1) * CH])
                    tiles.append(wv)
                return tiles

            wvs_next = load_wv(0)
            for dhc in range(DIM // CH):
                wvs = wvs_next
                if dhc + 1 < DIM // CH:
                    wvs_next = load_wv(dhc + 1)
                for tt in range(RPC // 128):
                    ps = psV.tile([128, CH], f32, name="psv", tag="V")
                    for dt_i in range(NDT):
                        nc.tensor.matmul(
                            ps[:],
                            xT[tt // 4][dt_i][:, (tt % 4) * 128:
                                              (tt % 4) * 128 + 128],
                            wvs[dt_i][:],
                            start=(dt_i == 0), stop=(dt_i == NDT - 1))
                    sb = vsb.tile([128, CH], bf16, name="vsbt", tag="vsbt")
                    nc.vector.scalar_tensor_tensor(
                        out=sb[:], in0=ps[:], scalar=1.0,
                        in1=bvb[:, dhc * CH:(dhc + 1) * CH],
                        op0=ALU.mult, op1=ALU.add)
                    # split the 512 v-dims into the two destination blocks
                    for half in range(2):
                        dest = 2 * dhc + half
                        nc.sync.dma_start(
                            out=v_send[dest * RPC + tt * 128:
                                       dest * RPC + (tt + 1) * 128, :],
                            in_=sb[:, half * (HPC * HD):
                                   (half + 1) * (HPC * HD)])

            nc.gpsimd.collective_compute(
                "AllToAll", mybir.AluOpType.bypass, replica_groups=rg,
                ins=[v_send.opt()], outs=[v_out.opt()])
            for m in range(NQKM // 2):
                qk_tile(m)
            nc.gpsimd.collective_compute(
                "AllToAll", mybir.AluOpType.bypass, replica_groups=rg,
                ins=[q_send.opt()], outs=[q_out.opt()])
            for m in range(NQKM // 2, 3 * NQKM // 4):
                qk_tile(m)
            nc.gpsimd.collective_compute(
                "AllToAll", mybir.AluOpType.bypass, replica_groups=rg,
                ins=[k_sendA.opt()], outs=[k_outA.opt()])
            for m in range(3 * NQKM // 4, NQKM):
                qk_tile(m)
            nc.gpsimd.collective_compute(
                "AllToAll", mybir.AluOpType.bypass, replica_groups=rg,
                ins=[k_sendB.opt()], outs=[k_outB.opt()])

        # Long-lived pools for the output projection: weights prefetch on
        # the idle gpsimd DMA queue DURING attention so the proj matmuls
        # start the moment attention (and the output AllToAlls) finish.
        # Created before the persistent qkv tiles to keep pool release LIFO.
        wpp = top.enter_context(tc.tile_pool(name="wpp", bufs=1))
        ocp = top.enter_context(tc.tile_pool(name="ocp", bufs=1))
        bcproj = top.enter_context(tc.tile_pool(name="bcproj", bufs=1))
        resp = top.enter_context(tc.tile_pool(name="resp", bufs=2))

        # persistent qkv storage for the attention phase (bf16)
        qT = [[None] * HPC for _ in range(B)]
        kT = [[None] * HPC for _ in range(B)]
        vS = [[None] * HPC for _ in range(B)]
        frees = []
        for b in range(B):
            for h in range(HPC):
                t1, f1 = tc.tile([128, S], bf16, name=f"qT{b}{h}")
                t2, f2 = tc.tile([128, S], bf16, name=f"kT{b}{h}")
                t3, f3 = tc.tile([128, S], bf16, name=f"vS{b}{h}")
                qT[b][h], kT[b][h], vS[b][h] = t1, t2, t3
                frees += [f1, f2, f3]

        # ================= Phase 2: head-parallel attention ===========
        with ExitStack() as p2:
            psG = p2.enter_context(tc.tile_pool(name="psG", bufs=2,
                                                space="PSUM"))
            psO = p2.enter_context(tc.tile_pool(name="psO", bufs=2,
                                                space="PSUM"))
            psS = p2.enter_context(tc.tile_pool(name="psS", bufs=2,
                                                space="PSUM"))
            ptp = p2.enter_context(tc.tile_pool(name="ptp", bufs=3))
            recp = p2.enter_context(tc.tile_pool(name="recp", bufs=2))
            otp = p2.enter_context(tc.tile_pool(name="otp", bufs=3))

            def load_qv(b):
                for h in range(HPC):
                    for sh in range(2):  # source half (token halves)
                        src = 2 * b + sh
                        r0 = src * (HPC * HD)
                        nc.sync.dma_start(
                            out=qT[b][h][:, sh * RPC:(sh + 1) * RPC],
                            in_=q_out[r0 + h * HD:r0 + (h + 1) * HD, :])
                    # v natural: [kv-in-tile 128, kv-tile x head-dim]
                    nc.sync.dma_start(
                        out=vS[b][h][:].rearrange("r (t c) -> r t c",
                                                  t=S // 128),
                        in_=v_out[b * S:(b + 1) * S,
                                  h * HD:(h + 1) * HD].rearrange(
                                      "(t r) c -> r t c", r=128))

            def load_k(b, h):
                k_src = k_outA if h == 0 else k_outB
                for sh in range(2):
                    src = 2 * b + sh
                    nc.sync.dma_start(
                        out=kT[b][h][:, sh * RPC:(sh + 1) * RPC],
                        in_=k_src[src * HD:(src + 1) * HD, :])

            state = {}  # (b,h,qc) -> (o_ps, s_sum)

            def front(b, h, qc, p0):
                ents = sched[qc]
                if p0 == 0:
                    state[(b, h, qc)] = (
                        psO.tile([128, CH], f32, name="o_ps", tag="O"),
                        psS.tile([128, CH], f32, name="s_sum", tag="Ssum"))
                n = min(2, len(ents) - p0)
                spsG = psG.tile([128, 2 * CH], f32, name="spsG", tag="G")
                pT = ptp.tile([128, 2 * CH], bf16, name="pT", tag="pT")
                g_lo = None
                for i in range(n):
                    (kc, j, q_lo, mid, c_lo, c_hi) = ents[p0 + i]
                    kv0 = kc * CH + j * SUB
                    off = i * CH
                    nc.tensor.matmul(
                        spsG[:, off + q_lo:off + CH],
                        kT[b][h][:, kv0:kv0 + SUB],
                        qT[b][h][:, qc * CH + q_lo:(qc + 1) * CH],
                        start=True, stop=True)
                    if mid >= 0:
                        nc.vector.tensor_add(
                            spsG[:, off + c_lo:off + c_hi],
                            spsG[:, off + c_lo:off + c_hi],
                            msk[mid][:, :c_hi - c_lo])
                    lo = off + q_lo
                    g_lo = lo if g_lo is None else min(g_lo, lo)
                nc.scalar.activation(
                    pT[:, g_lo:n * CH], spsG[:, g_lo:n * CH],
                    AF.Exp, scale=SCALE)
                return pT

            def back(b, h, qc, p0, pT):
                ents = sched[qc]
                last = len(ents) - 1
                o_ps, s_sum = state[(b, h, qc)]
                n = min(2, len(ents) - p0)
                for i in range(n):
                    ei = p0 + i
                    (kc, j, q_lo, mid, c_lo, c_hi) = ents[ei]
                    off = i * CH
                    nc.tensor.matmul(
                        s_sum[:, q_lo:CH], ones[:],
                        pT[:, off + q_lo:off + CH],
                        start=(ei == 0), stop=(ei == last))
                    kvt = kc * (CH // SUB) + j
                    nc.tensor.matmul(
                        o_ps[:, q_lo:CH],
                        vS[b][h][:, kvt * HD:(kvt + 1) * HD],
                        pT[:, off + q_lo:off + CH],
                        start=(ei == 0), stop=(ei == last))
                if p0 + n > last:  # query chunk complete
                    del state[(b, h, qc)]

                    # Deferred: issued after the NEXT unit's mask-add/exp so
                    # these DVE ops never delay the exp the tensor engine is
                    # waiting on (the DVE queue is strict FIFO).
                    def fin(o_ps=o_ps, s_sum=s_sum, b=b, h=h, qc=qc):
                        rec = recp.tile([128, CH], f32, name="rec",
                                        tag="rec")
                        # ~5x faster than reciprocal(); 18-bit accuracy is
                        # ample for bf16 and s_sum is a positive normal
                        nc.vector.reciprocal_approx_fast(rec[:], s_sum[:])
                        oT = otp.tile([128, CH], bf16, name="oT", tag="oT")
                        nc.vector.tensor_mul(oT[:], o_ps[:], rec[:])
                        dest = 2 * b + qc // 2
                        r0 = dest * (HPC * HD) + h * HD
                        nc.sync.dma_start(
                            out=a2a_in[qc % 2][r0:r0 + HD, :], in_=oT[:])
                    return fin
                return None

            def attn_section(items):
                # one software pipeline across all (b,h,qc) in the section
                units = []
                for (b, h, qcs) in items:
                    for qc in qcs:
                        for p0 in range(0, len(sched[qc]), 2):
                            units.append((b, h, qc, p0))
                pipe = []
                pending = []

                def flush():
                    while pending:
                        pending.pop(0)()

                for u in units:
                    if u[3] == 0:
                        # this front() reallocates PSUM state; the deferred
                        # reads of the old buffers must be issued first
                        flush()
                    pT = front(*u)
                    if u[3] != 0:
                        flush()
                    pipe.append((u, pT))
                    if len(pipe) > 1:
                        (b, h, qc, p0), pTx = pipe.pop(0)
                        f = back(b, h, qc, p0, pTx)
                        if f is not None:
                            pending.append(f)
                while pipe:
                    (b, h, qc, p0), pTx = pipe.pop(0)
                    f = back(b, h, qc, p0, pTx)
                    if f is not None:
                        pending.append(f)
                flush()

            # Prefetch the output-projection weights + bias broadcast on
            # the gpsimd queue FIRST (no dependencies -- issues instantly);
            # the collective-gated q/v loads follow behind them.
            wps = []  # [ot][nchunk] -> [128, CH]
            for ot in range(NDT):
                row = []
                for nchunk in range(DIM // CH):
                    wp = wpp.tile([128, CH], bf16, name=f"wp{ot}_{nchunk}",
                                  tag=f"wp{ot}_{nchunk}")
                    nc.gpsimd.dma_start(
                        out=wp[:],
                        in_=wproj_ext[ot * 128:(ot + 1) * 128,
                                      nchunk * CH:(nchunk + 1) * CH])
                    row.append(wp)
                wps.append(row)
            bpb = bcproj.tile([128, DIM], bf16, name="bpb", tag="bpb")
            nc.gpsimd.partition_broadcast(bpb[:], bp1[:])

            for b in range(B):
                load_qv(b)
                load_k(b, 0)
            for b in range(B):
                load_k(b, 1)

            oc = [[None] * NDT for _ in range(2)]

            def load_oc(half, eng):
                for ot in range(NDT):
                    t = ocp.tile([128, RPC // 2], bf16,
                                 name=f"oc{half}_{ot}", tag=f"oc_{ot}")
                    eng.dma_start(
                        out=t[:],
                        in_=a2a_out[half][ot * 128:(ot + 1) * 128, :])
                    oc[half][ot] = t

            attn_section([(b, 0, range(NSC)) for b in range(B)])
            # second head: even query chunks first so the first output
            # AllToAll launches while the odd chunks still compute
            attn_section([(b, 1, (0, 2)) for b in range(B)])
            nc.gpsimd.collective_compute(
                "AllToAll", mybir.AluOpType.bypass, replica_groups=rg,
                ins=[a2a_in[0].opt()], outs=[a2a_out[0].opt()])
            load_oc(0, nc.sync)
            attn_section([(b, 1, (1, 3)) for b in range(B)])
            nc.gpsimd.collective_compute(
                "AllToAll", mybir.AluOpType.bypass, replica_groups=rg,
                ins=[a2a_in[1].opt()], outs=[a2a_out[1].opt()])

        for f in reversed(frees):
            f()

        # ================= Phase 4: output projection =================
        with ExitStack() as p4:
            psD = p4.enter_context(tc.tile_pool(name="psD", bufs=8,
                                                space="PSUM"))
            for half in range(2):
                if half == 1:
                    # issued after half-0's matmuls so the tag-based WAR
                    # dependency lets these loads flow in behind them; the
                    # scalar queue is idle here (sync is busy with stores)
                    load_oc(1, nc.scalar)
                for rt in range(RPC // 2 // 128):
                    grow = half * (RPC // 2) + rt * 128
                    # nchunk-outer: each output chunk's accumulation closes
                    # a quarter-early, so bias-adds and stores drain behind
                    # the remaining matmuls instead of serializing at the end
                    for nchunk in range(DIM // CH):
                        ps = psD.tile([128, CH], f32, name="pso", tag="D")
                        for ot in range(NDT):
                            nc.tensor.matmul(
                                ps,
                                oc[half][ot][:, rt * 128:(rt + 1) * 128],
                                wps[ot][nchunk][:],
                                start=(ot == 0), stop=(ot == NDT - 1))
                        res = resp.tile([128, CH], bf16, name="res",
                                        tag="res")
                        nc.vector.scalar_tensor_tensor(
                            out=res[:], in0=ps[:], scalar=1.0,
                            in1=bpb[:, nchunk * CH:(nchunk + 1) * CH],
                            op0=ALU.mult, op1=ALU.add)
                        nc.sync.dma_start(
                            out=out_ext[grow:grow + 128,
                                        nchunk * CH:(nchunk + 1) * CH],
                            in_=res[:])

    nc.compile()
    return nc


def _get_program(sched, n_real, mask_widths):
    key = (str(sched), tuple(mask_widths))
    if key not in _prog_cache:
        _prog_cache[key] = _build_program(sched, n_real, mask_widths)
    return _prog_cache[key]


def kernel(x=None, mask=None, Wqkv=None, bqkv=None, Wproj=None, bproj=None,
           start_pos=0, **_unused):
    from concourse.bass_utils import run_bass_kernel_spmd

    x = np.ascontiguousarray(np.asarray(x, dtype=np.float32).reshape(TOK, DIM))
    mask = np.asarray(mask, dtype=np.float32)
    Wqkv = np.asarray(Wqkv, dtype=np.float32)
    bqkv = np.asarray(bqkv, dtype=np.float32)
    Wproj = np.asarray(Wproj, dtype=np.float32)
    bproj = np.asarray(bproj, dtype=np.float32)

    sched, mask_pack, widths, n_real = _analyze_mask(mask)
    nc = _get_program(sched, n_real, widths)

    # q/k weight columns reordered dest-major: for each destination core
    # d: [q_{2d}, q_{2d+1}, k_{2d}, k_{2d+1}]
    qk_cols = []
    for d in range(NCORES):
        for hh in (HPC * d, HPC * d + 1):
            qk_cols.append((hh * HD, (hh + 1) * HD))          # q
    for hl in range(HPC):
        for d in range(NCORES):
            hh = HPC * d + hl
            qk_cols.append((DIM + hh * HD, DIM + (hh + 1) * HD))  # k
    wqk = np.concatenate([Wqkv[:, a:b] for a, b in qk_cols], axis=1)
    bqk = np.concatenate([bqkv[a:b] for a, b in qk_cols])
    wv = Wqkv[:, 2 * DIM:]
    bv = bqkv[2 * DIM:]

    shared = {
        "wqk": np.ascontiguousarray(wqk.astype(_BF16)),
        "wv": np.ascontiguousarray(wv.astype(_BF16)),
        "bqk": np.ascontiguousarray(bqk.reshape(-1, 1)),
        "bv": np.ascontiguousarray(bv.reshape(1, DIM)),
        "maskt": mask_pack,
        "wproj": np.ascontiguousarray(Wproj.astype(_BF16)),
        "bproj": np.ascontiguousarray(bproj.reshape(1, DIM).astype(_BF16)),
    }
    xb = x.astype(_BF16)
    in_maps = []
    for c in range(NCORES):
        m = dict(shared)
        # host-side transpose: device loads x^T tiles with plain DMAs
        m["xsT"] = np.ascontiguousarray(xb[c * RPC:(c + 1) * RPC].T)
        in_maps.append(m)

    import os
    kw = {}
    if os.environ.get("KERNEL_TRACE"):
        kw["trace"] = True
    res = run_bass_kernel_spmd(nc, in_maps, core_ids=list(range(NCORES)), **kw)
    globals()["LAST_RUN"] = res
    if getattr(res, "exec_time_ns", None):
        print(f"HW exec time: {res.exec_time_ns} ns")
    outs = [res.results[c]["out"].astype(np.float32) for c in range(NCORES)]
    full = np.concatenate(outs, axis=0).reshape(B, S, DIM)
    return full

